# revision 41
# baseline (speedup 1.0000x reference)
"""DKVMN forward kernel for 8 Trainium2 NeuronCores (v3).

Data-parallel over batch: B=128 -> 16 per core, split into 2 groups of
8 rows. Per-group state v [d=128 partitions, (m,b)=50*8=400 free] bf16.
(m,b) column order (col = m*8 + b) makes the e/a gate broadcasts have
stride-1 last dims so DVE tensor_tensor runs in 2x 16-bit mode.

v3 rebalance vs v2 (which was Vector-saturated at 87% with 8 DVE ops
per slot, each with ~153ns fixed cost):
  - z / t1 / v' emitted as single full-width [D,400] ops (no halves):
    4 DVE ops per slot instead of 8.
  - t2 = w*bc(a) moved to the idle Pool engine (nc.gpsimd).
  - update reassociated: p = v - t1 (DVE), v' = p + t2 (DVE).
  - w broadcast for both groups of a step done as a pair: 2 matmuls
    into one 2-bank PSUM tile, ONE wide ACT copy [D,800] f32->bf16
    (saves one ACT fixed cost + one LDWEIGHTS per step).
  - W1q@qe prefetched in blocks of 8 steps (one 128-col matmul into a
    PSUM block; per-slot read matmuls accumulate into their 8-col
    slice) instead of 200 tiny per-slot matmuls.
  - read matmul merged to one 400-row matmul per slot (stride-0 PSUM
    output AP accumulates the 50 m-slices).

Slot schedule stays anti-phase (0, 2, 1, 4, 3, ...): group 0 runs one
step ahead of group 1 so each group's gate latency hides under the
other group's DVE/Pool stretch.
"""

import os
import numpy as np
import ml_dtypes
from contextlib import ExitStack

import concourse.bass as bass
import concourse.bacc as bacc
import concourse.mybir as mybir
import concourse.tile as tile
import concourse.bass_utils as bass_utils
from concourse.masks import make_identity

B, S, M, D, NQ = 128, 100, 50, 128, 10000
NCORES = 8
BC = B // NCORES          # 16 batch rows per core
GB = 8                    # rows per group
GW = M * GB               # 400 group state width
NQTILES = (S * BC + 127) // 128   # 13 gather tiles
QCOLS = NQTILES * 128     # 1664
HQB = 8                   # steps per W1q@qe prefetch block

F32 = mybir.dt.float32
BF16 = mybir.dt.bfloat16
I32 = mybir.dt.int32
AF = mybir.ActivationFunctionType
OP = mybir.AluOpType
AX = mybir.AxisListType

_CACHE = {}


def _build_program():
    if "nc" in _CACHE:
        return _CACHE["nc"]

    nc = bacc.Bacc("TRN2", target_bir_lowering=False, debug=False,
                   enable_asserts=False, num_devices=NCORES)

    dram_in = {}
    for name, shape, dt in [
        ("qtb", [D, QCOLS], BF16),
        ("qlast", [D, BC], BF16),
        ("kTb", [D, M], BF16),
        ("w1r", [D, D], BF16), ("w1q", [D, D], BF16),
        ("w2er", [D, D], BF16), ("w2ad", [D, D], BF16),
        ("b1", [D, 1], F32), ("eb", [D, 1], F32), ("ab", [D, 1], F32),
        ("ow1r", [D, D], BF16), ("ow1q", [D, D], BF16),
        ("ob1", [D, 1], F32), ("ow2", [D, 1], BF16), ("ob2", [1, 1], F32),
    ]:
        dram_in[name] = nc.dram_tensor(name, shape, dt, kind="ExternalInput").ap()
    pred_out = nc.dram_tensor("pred", [1, BC], F32, kind="ExternalOutput").ap()

    with tile.TileContext(nc) as tc, ExitStack() as ctx:
        persist = ctx.enter_context(tc.tile_pool(name="persist", bufs=1))

        # ---- persistent SBUF tiles ----
        kTb = persist.tile([D, M], BF16, tag="kTb")
        w1r = persist.tile([D, D], BF16, tag="w1r")
        w1q = persist.tile([D, D], BF16, tag="w1q")
        w2er = persist.tile([D, D], BF16, tag="w2er")
        w2ad = persist.tile([D, D], BF16, tag="w2ad")
        b1 = persist.tile([D, 1], F32, tag="b1")
        eb = persist.tile([D, 1], F32, tag="eb")
        ab = persist.tile([D, 1], F32, tag="ab")
        ow1r = persist.tile([D, D], BF16, tag="ow1r")
        ow1q = persist.tile([D, D], BF16, tag="ow1q")
        ob1 = persist.tile([D, 1], F32, tag="ob1")
        ow2 = persist.tile([D, 1], BF16, tag="ow2")
        ob2 = persist.tile([1, 1], F32, tag="ob2")
        ident = persist.tile([128, 128], F32, tag="ident")
        identb = persist.tile([128, 128], BF16, tag="identb")
        qlast = persist.tile([D, BC], BF16, tag="qlast")
        qTb = persist.tile([D, QCOLS], BF16, tag="qTb")
        attn = persist.tile([S, 2 * GW], F32, tag="attn")
        attnb = persist.tile([S, 2 * GW], BF16, tag="attnb")
        vpp = [[persist.tile([D, GW], BF16, name=f"v{g}p{p}", tag=f"v{g}p{p}")
                for p in (0, 1)] for g in (0, 1)]

        # DMA priority: kTb first (phase 2 needs it with qtb), then qtb
        # split across the three DMA-capable queues, then the scan
        # weights, and the final-head weights (needed ~400us in) last.
        nc.sync.dma_start(kTb[:], dram_in["kTb"][:])
        TH = QCOLS // 3
        nc.sync.dma_start(qTb[:, 0:TH], dram_in["qtb"][:, 0:TH])
        nc.scalar.dma_start(qTb[:, TH:2 * TH], dram_in["qtb"][:, TH:2 * TH])
        nc.gpsimd.dma_start(qTb[:, 2 * TH:QCOLS],
                            dram_in["qtb"][:, 2 * TH:QCOLS])
        for i, (nm, t) in enumerate([
                ("w1r", w1r), ("w1q", w1q),
                ("w2er", w2er), ("w2ad", w2ad), ("b1", b1),
                ("eb", eb), ("ab", ab)]):
            eng = (nc.scalar, nc.gpsimd)[i % 2]
            eng.dma_start(t[:], dram_in[nm][:])
        for i, (nm, t) in enumerate([
                ("qlast", qlast), ("ow1r", ow1r),
                ("ow1q", ow1q), ("ob1", ob1), ("ow2", ow2),
                ("ob2", ob2)]):
            eng = (nc.scalar, nc.gpsimd)[i % 2]
            eng.dma_start(t[:], dram_in[nm][:])
        make_identity(nc, ident[:])
        nc.vector.tensor_copy(identb[:], ident[:])
        nc.vector.memset(vpp[0][0][:], 0.0)
        nc.vector.memset(vpp[1][0][:], 0.0)

        # ---- phase 2: scores + softmax -> attn[s, (b,m)] f32 ----
        with tc.tile_pool(name="spsum", bufs=4, space="PSUM") as spsum:
            for b in range(BC):
                sc = spsum.tile([S, M], F32, tag="sc")
                qTsl = qTb[:, b:S * BC:BC]        # [128, 100] strided (s,b)
                nc.tensor.matmul(sc[:], qTsl, kTb[:], start=True, stop=True)
                if b % 2 == 0:
                    nc.vector.tensor_copy(attn[:, b * M:(b + 1) * M], sc[:])
                else:
                    nc.scalar.copy(attn[:, b * M:(b + 1) * M], sc[:])

        with tc.tile_pool(name="smx", bufs=1) as smx:
            a3 = attn[:].rearrange("p (b m) -> p b m", b=BC)
            # No max-subtraction: scores are O(1) here (xavier embeddings,
            # D=128), so raw exp is safe in f32.
            nc.scalar.activation(attn[:], attn[:], AF.Exp)
            sm = smx.tile([S, BC], F32, tag="sm")
            nc.vector.tensor_reduce(sm[:], a3, axis=AX.X, op=OP.add)
            rec = smx.tile([S, BC], F32, tag="rec")
            nc.vector.reciprocal(rec[:], sm[:])
            # normalize fused with the (b,m)->(m,b) bf16 reshuffle
            src = attn[:].rearrange("p (g b m) -> p g b m", g=2, b=GB)
            dst = attnb[:].rearrange("p (g m b) -> p g b m", g=2, m=M)
            recb = rec[:].rearrange("p (g b) -> p g b", g=2)[:, :, :, None] \
                .broadcast_to([S, 2, GB, M])
            nc.vector.tensor_tensor(dst, src, recb, op=OP.mult)
            # Preload the Tanh/Sigmoid activation table now (dead time)
            # instead of stalling the scan's first gate for ~1.3us.
            dmy = smx.tile([1, 1], F32, tag="dmy")
            nc.scalar.activation(dmy[:], rec[0:1, 0:1], AF.Tanh)

        # ---- phase 3: the scan ----
        # PSUM banks: wps 4 bufs x 1 bank + hqp 2 x 1 + mlpp 2 x 1 = 8.
        with tc.tile_pool(name="wps", bufs=4, space="PSUM") as wps, \
             tc.tile_pool(name="hqp", bufs=2, space="PSUM") as hqp, \
             tc.tile_pool(name="mlp", bufs=2, space="PSUM") as mlpp, \
             tc.tile_pool(name="w8p", bufs=8) as w8p, \
             tc.tile_pool(name="zp", bufs=4) as zp, \
             tc.tile_pool(name="wide", bufs=9) as wide, \
             tc.tile_pool(name="small", bufs=16) as small:

            NS = 2 * S  # slots
            wtile = [None] * NS     # per-slot w_sb bf16 slice APs
            hqt = {}                # step-block k0 -> PSUM tile
            state = [None] * NS     # (z, w, e, a) per slot
            wcopy_q = []            # slots whose w copy is pending

            wpsum = [None] * NS     # per-slot PSUM broadcast tiles

            def emit_w_mm(s):
                # One slot's w broadcast: one-hot row select via PE into a
                # 1-bank PSUM tile. The bf16 ACT copy (emit_w_copy) is
                # emitted a couple of iterations later so the ~585ns copy
                # lump never stalls a just-ready gate for long.
                t, g = s // 2, s % 2
                sel = identb[0:S, t:t + 1].broadcast_to([S, D])
                wp = wps.tile([D, GW], F32, tag="wp")
                nc.tensor.matmul(wp[:], sel, attnb[:, g * GW:(g + 1) * GW],
                                 start=True, stop=True)
                wpsum[s] = wp

            def emit_w_copy(s):
                w8 = w8p.tile([D, GW], BF16, tag="w8")
                nc.scalar.copy(w8[:], wpsum[s][:])
                wpsum[s] = None
                wtile[s] = w8[:]

            def emit_hq_block(k0):
                # W1q @ qe for steps [k0, k0+HQB): one 128-col matmul.
                # Per-slot read matmuls later accumulate into 8-col
                # slices (start=False), then ACT tanh reads the slice.
                ke = min(k0 + HQB, S)
                hq = hqp.tile([D, HQB * BC], F32, tag="hq")
                nc.tensor.matmul(hq[:, 0:(ke - k0) * BC], w1q[:],
                                 qTb[:, k0 * BC:ke * BC],
                                 start=True, stop=False,
                                 skip_group_check=True)
                hqt[k0] = hq

            def emit_read_gates(s):
                t, g = s // 2, s % 2
                w = wtile[s]
                vcur = vpp[g][t % 2]
                k0 = (t // HQB) * HQB
                hq = hqt[k0]
                hsl = hq[:, (t - k0) * BC + g * GB:(t - k0) * BC + (g + 1) * GB]
                # z full-width, then one fused read matmul:
                # W1r @ read = sum_m (W1r @ z[:,m]) via stride-0 PSUM output
                # AP (start=False accumulates by address) on top of the
                # prefetched W1q@qe slice.
                z = zp.tile([D, GW], BF16, tag="z")
                nc.vector.tensor_tensor(z[:], vcur[:], w, op=OP.mult)
                hbc = hsl[:, None, :].broadcast_to([D, M, GB])
                nc.tensor.matmul(
                    hbc, w1r[:],
                    z[:].rearrange("p (m b) -> p m b", m=M),
                    start=False, stop=True, skip_group_check=True)
                th = small.tile([D, GB], BF16, tag="th")
                nc.scalar.activation(th[:], hsl, AF.Tanh, bias=b1[:])
                # e and a get separate PSUM tiles (same tag ring): a shared
                # tile makes the second matmul wait on the first gate's ACT
                # read (tile-granular WAR), adding ~500ns to the chain.
                eps = mlpp.tile([D, GB], F32, tag="ea")
                nc.tensor.matmul(eps[:], w2er[:], th[:], start=True,
                                 stop=True)
                e = small.tile([D, GB], BF16, tag="e")
                nc.scalar.activation(e[:], eps[:], AF.Sigmoid, bias=eb[:])
                aps = mlpp.tile([D, GB], F32, tag="ea")
                nc.tensor.matmul(aps[:], w2ad[:], th[:], start=True,
                                 stop=True)
                a = small.tile([D, GB], BF16, tag="a")
                nc.scalar.activation(a[:], aps[:], AF.Tanh, bias=ab[:])
                # prefetched w copy in the ACT-idle window after the gates
                if wcopy_q:
                    emit_w_copy(wcopy_q.pop(0))
                state[s] = (z, w, e, a)

            def emit_update(s):
                t, g = s // 2, s % 2
                z, w, e, a = state[s]
                vcur, vnext = vpp[g][t % 2], vpp[g][(t + 1) % 2]
                # All DVE, ordered t1, p, t2, v': t1/p depend only on e
                # (which lands ~270ns before a), and by the time p is done
                # a has long arrived, so the queue never stalls mid-tail.
                # (Putting any of these on Pool was tried three ways and
                # always regressed: Pool's launch+sem latency is poison
                # anywhere near the chain.)
                ebc = e[:, None, :].broadcast_to([D, M, GB])
                t1 = wide.tile([D, GW], BF16, tag="t1")
                nc.vector.tensor_tensor(
                    t1[:].rearrange("p (m b) -> p m b", m=M),
                    z[:].rearrange("p (m b) -> p m b", m=M),
                    ebc, op=OP.mult)
                p = wide.tile([D, GW], BF16, tag="p")
                nc.vector.tensor_tensor(p[:], vcur[:], t1[:], op=OP.subtract)
                abc = a[:, None, :].broadcast_to([D, M, GB])
                t2 = wide.tile([D, GW], BF16, tag="t2")
                nc.vector.tensor_tensor(
                    t2[:].rearrange("p (m b) -> p m b", m=M),
                    w.rearrange("p (m b) -> p m b", m=M),
                    abc, op=OP.mult)
                nc.vector.tensor_tensor(vnext[:], p[:], t2[:], op=OP.add)
                state[s] = None

            # Anti-phase slot order (0, 2, 1, 4, 3, ...): group 0 runs one
            # step ahead of group 1 so each group's gate latency is hidden
            # under the other group's DVE stretch instead of bunching.
            order = [0] + [x for k in range(1, S) for x in (2 * k, 2 * k - 1)] \
                + [NS - 1]
            next_hq = [0]

            def ensure_hq(s):
                while next_hq[0] <= s // 2 and next_hq[0] < S:
                    emit_hq_block(next_hq[0])
                    next_hq[0] += HQB

            ensure_hq(order[0])
            for j in range(4):
                emit_w_mm(order[j])
            for j in range(2):
                emit_w_copy(order[j])
            for i, s in enumerate(order):
                if s >= 2:
                    emit_update(s - 2)
                if i + 2 < NS:
                    ensure_hq(order[i + 2])
                emit_read_gates(s)
                if i + 4 < NS:
                    emit_w_mm(order[i + 4])
                if i + 2 < NS:
                    wcopy_q.append(order[i + 2])
            while wcopy_q:
                emit_w_copy(wcopy_q.pop(0))
            emit_update(NS - 2)
            emit_update(NS - 1)

            # ---- final prediction (uses w from t=S-1, v after last update) ----
            # Final read via the same PE identity-accumulate trick as the
            # per-step read (stride-0 PSUM out AP sums the 50 m-slices);
            # beats two serial 840ns DVE tensor_reduces.
            rfps = mlpp.tile([D, BC], F32, tag="ea")
            nc.vector.memset(rfps[:], 0.0)
            for g in (0, 1):
                wf = wtile[2 * (S - 1) + g]
                zfin = zp.tile([D, GW], BF16, tag="z")
                nc.vector.tensor_tensor(zfin[:], vpp[g][S % 2][:], wf,
                                        op=OP.mult)
                rbc = rfps[:, None, g * GB:(g + 1) * GB] \
                    .broadcast_to([D, M, GB])
                nc.tensor.matmul(rbc, identb[:],
                                 zfin[:].rearrange("p (m b) -> p m b", m=M),
                                 start=False, stop=(g == 1),
                                 skip_group_check=True)
            readF = small.tile([D, BC], BF16, tag="readF")
            nc.scalar.copy(readF[:], rfps[:])
            h2ps = mlpp.tile([D, BC], F32, tag="ea")
            nc.tensor.matmul(h2ps[:], ow1r[:], readF[:], start=True, stop=False)
            nc.tensor.matmul(h2ps[:], ow1q[:], qlast[:], start=False, stop=True)
            h2 = small.tile([D, BC], BF16, tag="h2")
            nc.scalar.activation(h2[:], h2ps[:], AF.Relu, bias=ob1[:])
            pps = mlpp.tile([D, BC], F32, tag="ea")
            nc.tensor.matmul(pps[0:1, 0:BC], ow2[:], h2[:], start=True,
                             stop=True)
            ps = small.tile([1, BC], F32, tag="pred")
            nc.scalar.activation(ps[:], pps[0:1, 0:BC], AF.Sigmoid, bias=ob2[:])
            nc.sync.dma_start(pred_out[:], ps[:])

    nc.compile()
    _CACHE["nc"] = nc
    return nc


def _host_inputs(inputs):
    """Per-core input maps from the full problem inputs."""
    q = np.asarray(inputs["question_seq"]).astype(np.int64)
    emb = np.ascontiguousarray(np.asarray(inputs["emb"], dtype=np.float32))
    key_matrix = np.asarray(inputs["key_matrix"], dtype=np.float32)
    vu_w1 = np.asarray(inputs["vu_w1"], dtype=np.float32)
    vu_b1 = np.asarray(inputs["vu_b1"], dtype=np.float32)
    vu_w2 = np.asarray(inputs["vu_w2"], dtype=np.float32)
    vu_b2 = np.asarray(inputs["vu_b2"], dtype=np.float32)
    er_w = np.asarray(inputs["er_w"], dtype=np.float32)
    er_b = np.asarray(inputs["er_b"], dtype=np.float32)
    ad_w = np.asarray(inputs["ad_w"], dtype=np.float32)
    ad_b = np.asarray(inputs["ad_b"], dtype=np.float32)
    out_w1 = np.asarray(inputs["out_w1"], dtype=np.float32)
    out_b1 = np.asarray(inputs["out_b1"], dtype=np.float32)
    out_w2 = np.asarray(inputs["out_w2"], dtype=np.float32)
    out_b2 = np.asarray(inputs["out_b2"], dtype=np.float32)

    w2er = (vu_w2.astype(np.float64) @ er_w.astype(np.float64)).astype(np.float32)
    w2ad = (vu_w2.astype(np.float64) @ ad_w.astype(np.float64)).astype(np.float32)
    ebf = (vu_b2.astype(np.float64) @ er_w.astype(np.float64) + er_b).astype(np.float32)
    abf = (vu_b2.astype(np.float64) @ ad_w.astype(np.float64) + ad_b).astype(np.float32)

    bf = ml_dtypes.bfloat16
    shared = {
        "kTb": np.ascontiguousarray(key_matrix.T).astype(bf),
        "w1r": np.ascontiguousarray(vu_w1[:D]).astype(bf),
        "w1q": np.ascontiguousarray(vu_w1[D:]).astype(bf),
        "w2er": w2er.astype(bf), "w2ad": w2ad.astype(bf),
        "b1": vu_b1.reshape(D, 1), "eb": ebf.reshape(D, 1), "ab": abf.reshape(D, 1),
        "ow1r": np.ascontiguousarray(out_w1[:D]).astype(bf),
        "ow1q": np.ascontiguousarray(out_w1[D:]).astype(bf),
        "ob1": out_b1.reshape(D, 1),
        "ow2": np.ascontiguousarray(out_w2.reshape(D, 1)).astype(bf),
        "ob2": out_b2.reshape(1, 1),
    }
    in_maps = []
    for c in range(NCORES):
        qc = q[c * BC:(c + 1) * BC, :]          # [BC, S]
        idxs = qc.T.reshape(-1)                  # n = s*BC + b order
        qg = emb[idxs]                           # [S*BC, D]
        qtb = np.zeros((D, QCOLS), np.float32)
        qtb[:, :S * BC] = qg.T
        m = dict(shared)
        m["qtb"] = qtb.astype(bf)
        m["qlast"] = np.ascontiguousarray(qg[(S - 1) * BC:, :].T).astype(bf)
        in_maps.append(m)
    return in_maps


def _install_ntff_shim():
    # Optional: enables NTFF hardware profiling under axon when tracing is
    # requested. Harmless no-op if the pieces are missing.
    import types, sys
    if "antenv.axon_hooks" in sys.modules:
        return
    try:
        import antenv
        from trn_agent_boot.trn_boot import _ntff_profile_via_ctypes
        hook = _ntff_profile_via_ctypes("/opt/axon/libaxon_pjrt.so")
        mod = types.ModuleType("antenv.axon_hooks")
        state = {"hook": hook}
        mod.get_axon_ntff_profile_hook = lambda: state["hook"]
        mod.set_axon_ntff_profile_hook = lambda h: state.update(hook=h)
        sys.modules["antenv.axon_hooks"] = mod
        antenv.axon_hooks = mod
    except Exception:
        pass


def kernel(**inputs) -> np.ndarray:
    if bool(int(os.environ.get("DKVMN_TRACE", "0"))):
        _install_ntff_shim()
    nc = _build_program()
    in_maps = _host_inputs(inputs)
    res = bass_utils.run_bass_kernel_spmd(
        nc, in_maps, core_ids=list(range(NCORES)),
        trace=bool(int(os.environ.get("DKVMN_TRACE", "0"))),
    )
    _CACHE["last_results"] = res
    pred = np.concatenate([res.results[c]["pred"].reshape(BC) for c in range(NCORES)])
    return pred.astype(np.float32)


# revision 42
# speedup vs baseline: 1.0045x; 1.0045x over previous
"""DKVMN forward kernel for 8 Trainium2 NeuronCores (v3).

Data-parallel over batch: B=128 -> 16 per core, split into 2 groups of
8 rows. Per-group state v [d=128 partitions, (m,b)=50*8=400 free] bf16.
(m,b) column order (col = m*8 + b) makes the e/a gate broadcasts have
stride-1 last dims so DVE tensor_tensor runs in 2x 16-bit mode.

v3 rebalance vs v2 (which was Vector-saturated at 87% with 8 DVE ops
per slot, each with ~153ns fixed cost):
  - z / t1 / v' emitted as single full-width [D,400] ops (no halves):
    4 DVE ops per slot instead of 8.
  - t2 = w*bc(a) moved to the idle Pool engine (nc.gpsimd).
  - update reassociated: p = v - t1 (DVE), v' = p + t2 (DVE).
  - w broadcast for both groups of a step done as a pair: 2 matmuls
    into one 2-bank PSUM tile, ONE wide ACT copy [D,800] f32->bf16
    (saves one ACT fixed cost + one LDWEIGHTS per step).
  - W1q@qe prefetched in blocks of 8 steps (one 128-col matmul into a
    PSUM block; per-slot read matmuls accumulate into their 8-col
    slice) instead of 200 tiny per-slot matmuls.
  - read matmul merged to one 400-row matmul per slot (stride-0 PSUM
    output AP accumulates the 50 m-slices).

Slot schedule stays anti-phase (0, 2, 1, 4, 3, ...): group 0 runs one
step ahead of group 1 so each group's gate latency hides under the
other group's DVE/Pool stretch.
"""

import os
import numpy as np
import ml_dtypes
from contextlib import ExitStack

import concourse.bass as bass
import concourse.bacc as bacc
import concourse.mybir as mybir
import concourse.tile as tile
import concourse.bass_utils as bass_utils
from concourse.masks import make_identity

B, S, M, D, NQ = 128, 100, 50, 128, 10000
NCORES = 8
BC = B // NCORES          # 16 batch rows per core
GB = 8                    # rows per group
GW = M * GB               # 400 group state width
NQTILES = (S * BC + 127) // 128   # 13 gather tiles
QCOLS = NQTILES * 128     # 1664
HQB = 8                   # steps per W1q@qe prefetch block

F32 = mybir.dt.float32
BF16 = mybir.dt.bfloat16
I32 = mybir.dt.int32
AF = mybir.ActivationFunctionType
OP = mybir.AluOpType
AX = mybir.AxisListType

_CACHE = {}


def _build_program():
    if "nc" in _CACHE:
        return _CACHE["nc"]

    nc = bacc.Bacc("TRN2", target_bir_lowering=False, debug=False,
                   enable_asserts=False, num_devices=NCORES)

    dram_in = {}
    for name, shape, dt in [
        ("qtb", [D, QCOLS], BF16),
        ("qlast", [D, BC], BF16),
        ("kTb", [D, M], BF16),
        ("w1r", [D, D], BF16), ("w1q", [D, D], BF16),
        ("w2er", [D, D], BF16), ("w2ad", [D, D], BF16),
        ("b1", [D, 1], F32), ("eb", [D, 1], F32), ("ab", [D, 1], F32),
        ("ow1r", [D, D], BF16), ("ow1q", [D, D], BF16),
        ("ob1", [D, 1], F32), ("ow2", [D, 1], BF16), ("ob2", [1, 1], F32),
    ]:
        dram_in[name] = nc.dram_tensor(name, shape, dt, kind="ExternalInput").ap()
    pred_out = nc.dram_tensor("pred", [1, BC], F32, kind="ExternalOutput").ap()

    with tile.TileContext(nc) as tc, ExitStack() as ctx:
        persist = ctx.enter_context(tc.tile_pool(name="persist", bufs=1))

        # ---- persistent SBUF tiles ----
        kTb = persist.tile([D, M], BF16, tag="kTb")
        w1r = persist.tile([D, D], BF16, tag="w1r")
        w1q = persist.tile([D, D], BF16, tag="w1q")
        w2er = persist.tile([D, D], BF16, tag="w2er")
        w2ad = persist.tile([D, D], BF16, tag="w2ad")
        b1 = persist.tile([D, 1], F32, tag="b1")
        eb = persist.tile([D, 1], F32, tag="eb")
        ab = persist.tile([D, 1], F32, tag="ab")
        ow1r = persist.tile([D, D], BF16, tag="ow1r")
        ow1q = persist.tile([D, D], BF16, tag="ow1q")
        ob1 = persist.tile([D, 1], F32, tag="ob1")
        ow2 = persist.tile([D, 1], BF16, tag="ow2")
        ob2 = persist.tile([1, 1], F32, tag="ob2")
        ident = persist.tile([128, 128], F32, tag="ident")
        identb = persist.tile([128, 128], BF16, tag="identb")
        qlast = persist.tile([D, BC], BF16, tag="qlast")
        qTb = persist.tile([D, QCOLS], BF16, tag="qTb")
        attn = persist.tile([S, 2 * GW], F32, tag="attn")
        attnb = persist.tile([S, 2 * GW], BF16, tag="attnb")
        vpp = [[persist.tile([D, GW], BF16, name=f"v{g}p{p}", tag=f"v{g}p{p}")
                for p in (0, 1)] for g in (0, 1)]

        # DMA priority: kTb first (phase 2 needs it with qtb), then qtb
        # split across the three DMA-capable queues, then the scan
        # weights, and the final-head weights (needed ~400us in) last.
        nc.sync.dma_start(kTb[:], dram_in["kTb"][:])
        TH = QCOLS // 3
        nc.sync.dma_start(qTb[:, 0:TH], dram_in["qtb"][:, 0:TH])
        nc.scalar.dma_start(qTb[:, TH:2 * TH], dram_in["qtb"][:, TH:2 * TH])
        nc.gpsimd.dma_start(qTb[:, 2 * TH:QCOLS],
                            dram_in["qtb"][:, 2 * TH:QCOLS])
        for i, (nm, t) in enumerate([
                ("w1r", w1r), ("w1q", w1q),
                ("w2er", w2er), ("w2ad", w2ad), ("b1", b1),
                ("eb", eb), ("ab", ab)]):
            eng = (nc.scalar, nc.gpsimd)[i % 2]
            eng.dma_start(t[:], dram_in[nm][:])
        for i, (nm, t) in enumerate([
                ("qlast", qlast), ("ow1r", ow1r),
                ("ow1q", ow1q), ("ob1", ob1), ("ow2", ow2),
                ("ob2", ob2)]):
            eng = (nc.scalar, nc.gpsimd)[i % 2]
            eng.dma_start(t[:], dram_in[nm][:])
        make_identity(nc, ident[:])
        nc.vector.tensor_copy(identb[:], ident[:])
        nc.vector.memset(vpp[0][0][:], 0.0)
        nc.vector.memset(vpp[1][0][:], 0.0)

        # ---- phase 2: scores + softmax -> attn[s, (b,m)] f32 ----
        with tc.tile_pool(name="spsum", bufs=4, space="PSUM") as spsum:
            for b in range(BC):
                sc = spsum.tile([S, M], F32, tag="sc")
                qTsl = qTb[:, b:S * BC:BC]        # [128, 100] strided (s,b)
                nc.tensor.matmul(sc[:], qTsl, kTb[:], start=True, stop=True)
                if b % 2 == 0:
                    nc.vector.tensor_copy(attn[:, b * M:(b + 1) * M], sc[:])
                else:
                    nc.scalar.copy(attn[:, b * M:(b + 1) * M], sc[:])

        with tc.tile_pool(name="smx", bufs=1) as smx:
            a3 = attn[:].rearrange("p (b m) -> p b m", b=BC)
            # No max-subtraction: scores are O(1) here (xavier embeddings,
            # D=128), so raw exp is safe in f32.
            nc.scalar.activation(attn[:], attn[:], AF.Exp)
            sm = smx.tile([S, BC], F32, tag="sm")
            nc.vector.tensor_reduce(sm[:], a3, axis=AX.X, op=OP.add)
            rec = smx.tile([S, BC], F32, tag="rec")
            nc.vector.reciprocal(rec[:], sm[:])
            # normalize fused with the (b,m)->(m,b) bf16 reshuffle
            src = attn[:].rearrange("p (g b m) -> p g b m", g=2, b=GB)
            dst = attnb[:].rearrange("p (g m b) -> p g b m", g=2, m=M)
            recb = rec[:].rearrange("p (g b) -> p g b", g=2)[:, :, :, None] \
                .broadcast_to([S, 2, GB, M])
            nc.vector.tensor_tensor(dst, src, recb, op=OP.mult)
            # Preload the Tanh/Sigmoid activation table now (dead time)
            # instead of stalling the scan's first gate for ~1.3us.
            dmy = smx.tile([1, 1], F32, tag="dmy")
            nc.scalar.activation(dmy[:], rec[0:1, 0:1], AF.Tanh)

        # ---- phase 3: the scan ----
        # PSUM banks: wps 4 bufs x 1 bank + hqp 2 x 1 + mlpp 2 x 1 = 8.
        with tc.tile_pool(name="wps", bufs=4, space="PSUM") as wps, \
             tc.tile_pool(name="hqp", bufs=2, space="PSUM") as hqp, \
             tc.tile_pool(name="mlp", bufs=2, space="PSUM") as mlpp, \
             tc.tile_pool(name="w8p", bufs=8) as w8p, \
             tc.tile_pool(name="zp", bufs=4) as zp, \
             tc.tile_pool(name="wide", bufs=9) as wide, \
             tc.tile_pool(name="small", bufs=16) as small:

            NS = 2 * S  # slots
            wtile = [None] * NS     # per-slot w_sb bf16 slice APs
            hqt = {}                # step-block k0 -> PSUM tile
            state = [None] * NS     # (z, w, e, a) per slot
            wcopy_q = []            # slots whose w copy is pending

            wpsum = [None] * NS     # per-slot PSUM broadcast tiles

            def emit_w_mm(s):
                # One slot's w broadcast: one-hot row select via PE into a
                # 1-bank PSUM tile. The bf16 ACT copy (emit_w_copy) is
                # emitted a couple of iterations later so the ~585ns copy
                # lump never stalls a just-ready gate for long.
                t, g = s // 2, s % 2
                sel = identb[0:S, t:t + 1].broadcast_to([S, D])
                wp = wps.tile([D, GW], F32, tag="wp")
                nc.tensor.matmul(wp[:], sel, attnb[:, g * GW:(g + 1) * GW],
                                 start=True, stop=True)
                wpsum[s] = wp

            def emit_w_copy(s):
                w8 = w8p.tile([D, GW], BF16, tag="w8")
                nc.scalar.copy(w8[:], wpsum[s][:])
                wpsum[s] = None
                wtile[s] = w8[:]

            def emit_hq_block(k0):
                # W1q @ qe for steps [k0, k0+HQB): one 128-col matmul.
                # Per-slot read matmuls later accumulate into 8-col
                # slices (start=False), then ACT tanh reads the slice.
                ke = min(k0 + HQB, S)
                hq = hqp.tile([D, HQB * BC], F32, tag="hq")
                nc.tensor.matmul(hq[:, 0:(ke - k0) * BC], w1q[:],
                                 qTb[:, k0 * BC:ke * BC],
                                 start=True, stop=False,
                                 skip_group_check=True)
                hqt[k0] = hq

            def emit_read_gates(s):
                t, g = s // 2, s % 2
                w = wtile[s]
                vcur = vpp[g][t % 2]
                k0 = (t // HQB) * HQB
                hq = hqt[k0]
                hsl = hq[:, (t - k0) * BC + g * GB:(t - k0) * BC + (g + 1) * GB]
                # z full-width, then one fused read matmul:
                # W1r @ read = sum_m (W1r @ z[:,m]) via stride-0 PSUM output
                # AP (start=False accumulates by address) on top of the
                # prefetched W1q@qe slice.
                z = zp.tile([D, GW], BF16, tag="z")
                nc.vector.tensor_tensor(z[:], vcur[:], w, op=OP.mult)
                hbc = hsl[:, None, :].broadcast_to([D, M, GB])
                nc.tensor.matmul(
                    hbc, w1r[:],
                    z[:].rearrange("p (m b) -> p m b", m=M),
                    start=False, stop=True, skip_group_check=True)
                th = small.tile([D, GB], BF16, tag="th")
                nc.scalar.activation(th[:], hsl, AF.Tanh, bias=b1[:])
                # The prefetched w copy goes right here in the Scalar
                # queue: while e-mm does its PE round trip (~430ns), ACT
                # would idle anyway, so the copy lump can't delay e-act by
                # much.
                if wcopy_q:
                    emit_w_copy(wcopy_q.pop(0))
                # e and a get separate PSUM tiles (same tag ring): a shared
                # tile makes the second matmul wait on the first gate's ACT
                # read (tile-granular WAR), adding ~500ns to the chain.
                eps = mlpp.tile([D, GB], F32, tag="ea")
                nc.tensor.matmul(eps[:], w2er[:], th[:], start=True,
                                 stop=True)
                e = small.tile([D, GB], BF16, tag="e")
                nc.scalar.activation(e[:], eps[:], AF.Sigmoid, bias=eb[:])
                aps = mlpp.tile([D, GB], F32, tag="ea")
                nc.tensor.matmul(aps[:], w2ad[:], th[:], start=True,
                                 stop=True)
                a = small.tile([D, GB], BF16, tag="a")
                nc.scalar.activation(a[:], aps[:], AF.Tanh, bias=ab[:])
                state[s] = (z, w, e, a)

            def emit_update(s):
                t, g = s // 2, s % 2
                z, w, e, a = state[s]
                vcur, vnext = vpp[g][t % 2], vpp[g][(t + 1) % 2]
                # All DVE, ordered t1, p, t2, v': t1/p depend only on e
                # (which lands ~270ns before a), and by the time p is done
                # a has long arrived, so the queue never stalls mid-tail.
                # (Putting any of these on Pool was tried three ways and
                # always regressed: Pool's launch+sem latency is poison
                # anywhere near the chain.)
                ebc = e[:, None, :].broadcast_to([D, M, GB])
                t1 = wide.tile([D, GW], BF16, tag="t1")
                nc.vector.tensor_tensor(
                    t1[:].rearrange("p (m b) -> p m b", m=M),
                    z[:].rearrange("p (m b) -> p m b", m=M),
                    ebc, op=OP.mult)
                p = wide.tile([D, GW], BF16, tag="p")
                nc.vector.tensor_tensor(p[:], vcur[:], t1[:], op=OP.subtract)
                abc = a[:, None, :].broadcast_to([D, M, GB])
                t2 = wide.tile([D, GW], BF16, tag="t2")
                nc.vector.tensor_tensor(
                    t2[:].rearrange("p (m b) -> p m b", m=M),
                    w.rearrange("p (m b) -> p m b", m=M),
                    abc, op=OP.mult)
                nc.vector.tensor_tensor(vnext[:], p[:], t2[:], op=OP.add)
                state[s] = None

            # Anti-phase slot order (0, 2, 1, 4, 3, ...): group 0 runs one
            # step ahead of group 1 so each group's gate latency is hidden
            # under the other group's DVE stretch instead of bunching.
            order = [0] + [x for k in range(1, S) for x in (2 * k, 2 * k - 1)] \
                + [NS - 1]
            next_hq = [0]

            def ensure_hq(s):
                while next_hq[0] <= s // 2 and next_hq[0] < S:
                    emit_hq_block(next_hq[0])
                    next_hq[0] += HQB

            ensure_hq(order[0])
            for j in range(5):
                emit_w_mm(order[j])
            for j in range(2):
                emit_w_copy(order[j])
            for i, s in enumerate(order):
                if s >= 2:
                    emit_update(s - 2)
                if i + 2 < NS:
                    ensure_hq(order[i + 2])
                emit_read_gates(s)
                if i + 5 < NS:
                    emit_w_mm(order[i + 5])
                if i + 2 < NS:
                    wcopy_q.append(order[i + 2])
            while wcopy_q:
                emit_w_copy(wcopy_q.pop(0))
            emit_update(NS - 2)
            emit_update(NS - 1)

            # ---- final prediction (uses w from t=S-1, v after last update) ----
            # Final read via the same PE identity-accumulate trick as the
            # per-step read (stride-0 PSUM out AP sums the 50 m-slices);
            # beats two serial 840ns DVE tensor_reduces.
            rfps = mlpp.tile([D, BC], F32, tag="ea")
            nc.vector.memset(rfps[:], 0.0)
            for g in (0, 1):
                wf = wtile[2 * (S - 1) + g]
                zfin = zp.tile([D, GW], BF16, tag="z")
                nc.vector.tensor_tensor(zfin[:], vpp[g][S % 2][:], wf,
                                        op=OP.mult)
                rbc = rfps[:, None, g * GB:(g + 1) * GB] \
                    .broadcast_to([D, M, GB])
                nc.tensor.matmul(rbc, identb[:],
                                 zfin[:].rearrange("p (m b) -> p m b", m=M),
                                 start=False, stop=(g == 1),
                                 skip_group_check=True)
            readF = small.tile([D, BC], BF16, tag="readF")
            nc.scalar.copy(readF[:], rfps[:])
            h2ps = mlpp.tile([D, BC], F32, tag="ea")
            nc.tensor.matmul(h2ps[:], ow1r[:], readF[:], start=True, stop=False)
            nc.tensor.matmul(h2ps[:], ow1q[:], qlast[:], start=False, stop=True)
            h2 = small.tile([D, BC], BF16, tag="h2")
            nc.scalar.activation(h2[:], h2ps[:], AF.Relu, bias=ob1[:])
            pps = mlpp.tile([D, BC], F32, tag="ea")
            nc.tensor.matmul(pps[0:1, 0:BC], ow2[:], h2[:], start=True,
                             stop=True)
            ps = small.tile([1, BC], F32, tag="pred")
            nc.scalar.activation(ps[:], pps[0:1, 0:BC], AF.Sigmoid, bias=ob2[:])
            nc.sync.dma_start(pred_out[:], ps[:])

    nc.compile()
    _CACHE["nc"] = nc
    return nc


def _host_inputs(inputs):
    """Per-core input maps from the full problem inputs."""
    q = np.asarray(inputs["question_seq"]).astype(np.int64)
    emb = np.ascontiguousarray(np.asarray(inputs["emb"], dtype=np.float32))
    key_matrix = np.asarray(inputs["key_matrix"], dtype=np.float32)
    vu_w1 = np.asarray(inputs["vu_w1"], dtype=np.float32)
    vu_b1 = np.asarray(inputs["vu_b1"], dtype=np.float32)
    vu_w2 = np.asarray(inputs["vu_w2"], dtype=np.float32)
    vu_b2 = np.asarray(inputs["vu_b2"], dtype=np.float32)
    er_w = np.asarray(inputs["er_w"], dtype=np.float32)
    er_b = np.asarray(inputs["er_b"], dtype=np.float32)
    ad_w = np.asarray(inputs["ad_w"], dtype=np.float32)
    ad_b = np.asarray(inputs["ad_b"], dtype=np.float32)
    out_w1 = np.asarray(inputs["out_w1"], dtype=np.float32)
    out_b1 = np.asarray(inputs["out_b1"], dtype=np.float32)
    out_w2 = np.asarray(inputs["out_w2"], dtype=np.float32)
    out_b2 = np.asarray(inputs["out_b2"], dtype=np.float32)

    w2er = (vu_w2.astype(np.float64) @ er_w.astype(np.float64)).astype(np.float32)
    w2ad = (vu_w2.astype(np.float64) @ ad_w.astype(np.float64)).astype(np.float32)
    ebf = (vu_b2.astype(np.float64) @ er_w.astype(np.float64) + er_b).astype(np.float32)
    abf = (vu_b2.astype(np.float64) @ ad_w.astype(np.float64) + ad_b).astype(np.float32)

    bf = ml_dtypes.bfloat16
    shared = {
        "kTb": np.ascontiguousarray(key_matrix.T).astype(bf),
        "w1r": np.ascontiguousarray(vu_w1[:D]).astype(bf),
        "w1q": np.ascontiguousarray(vu_w1[D:]).astype(bf),
        "w2er": w2er.astype(bf), "w2ad": w2ad.astype(bf),
        "b1": vu_b1.reshape(D, 1), "eb": ebf.reshape(D, 1), "ab": abf.reshape(D, 1),
        "ow1r": np.ascontiguousarray(out_w1[:D]).astype(bf),
        "ow1q": np.ascontiguousarray(out_w1[D:]).astype(bf),
        "ob1": out_b1.reshape(D, 1),
        "ow2": np.ascontiguousarray(out_w2.reshape(D, 1)).astype(bf),
        "ob2": out_b2.reshape(1, 1),
    }
    in_maps = []
    for c in range(NCORES):
        qc = q[c * BC:(c + 1) * BC, :]          # [BC, S]
        idxs = qc.T.reshape(-1)                  # n = s*BC + b order
        qg = emb[idxs]                           # [S*BC, D]
        qtb = np.zeros((D, QCOLS), np.float32)
        qtb[:, :S * BC] = qg.T
        m = dict(shared)
        m["qtb"] = qtb.astype(bf)
        m["qlast"] = np.ascontiguousarray(qg[(S - 1) * BC:, :].T).astype(bf)
        in_maps.append(m)
    return in_maps


def _install_ntff_shim():
    # Optional: enables NTFF hardware profiling under axon when tracing is
    # requested. Harmless no-op if the pieces are missing.
    import types, sys
    if "antenv.axon_hooks" in sys.modules:
        return
    try:
        import antenv
        from trn_agent_boot.trn_boot import _ntff_profile_via_ctypes
        hook = _ntff_profile_via_ctypes("/opt/axon/libaxon_pjrt.so")
        mod = types.ModuleType("antenv.axon_hooks")
        state = {"hook": hook}
        mod.get_axon_ntff_profile_hook = lambda: state["hook"]
        mod.set_axon_ntff_profile_hook = lambda h: state.update(hook=h)
        sys.modules["antenv.axon_hooks"] = mod
        antenv.axon_hooks = mod
    except Exception:
        pass


def kernel(**inputs) -> np.ndarray:
    if bool(int(os.environ.get("DKVMN_TRACE", "0"))):
        _install_ntff_shim()
    nc = _build_program()
    in_maps = _host_inputs(inputs)
    res = bass_utils.run_bass_kernel_spmd(
        nc, in_maps, core_ids=list(range(NCORES)),
        trace=bool(int(os.environ.get("DKVMN_TRACE", "0"))),
    )
    _CACHE["last_results"] = res
    pred = np.concatenate([res.results[c]["pred"].reshape(BC) for c in range(NCORES)])
    return pred.astype(np.float32)


# revision 43
# speedup vs baseline: 1.0049x; 1.0004x over previous
"""DKVMN forward kernel for 8 Trainium2 NeuronCores (v3).

Data-parallel over batch: B=128 -> 16 per core, split into 2 groups of
8 rows. Per-group state v [d=128 partitions, (m,b)=50*8=400 free] bf16.
(m,b) column order (col = m*8 + b) makes the e/a gate broadcasts have
stride-1 last dims so DVE tensor_tensor runs in 2x 16-bit mode.

v3 rebalance vs v2 (which was Vector-saturated at 87% with 8 DVE ops
per slot, each with ~153ns fixed cost):
  - z / t1 / v' emitted as single full-width [D,400] ops (no halves):
    4 DVE ops per slot instead of 8.
  - t2 = w*bc(a) moved to the idle Pool engine (nc.gpsimd).
  - update reassociated: p = v - t1 (DVE), v' = p + t2 (DVE).
  - w broadcast for both groups of a step done as a pair: 2 matmuls
    into one 2-bank PSUM tile, ONE wide ACT copy [D,800] f32->bf16
    (saves one ACT fixed cost + one LDWEIGHTS per step).
  - W1q@qe prefetched in blocks of 8 steps (one 128-col matmul into a
    PSUM block; per-slot read matmuls accumulate into their 8-col
    slice) instead of 200 tiny per-slot matmuls.
  - read matmul merged to one 400-row matmul per slot (stride-0 PSUM
    output AP accumulates the 50 m-slices).

Slot schedule stays anti-phase (0, 2, 1, 4, 3, ...): group 0 runs one
step ahead of group 1 so each group's gate latency hides under the
other group's DVE/Pool stretch.
"""

import os
import numpy as np
import ml_dtypes
from contextlib import ExitStack

import concourse.bass as bass
import concourse.bacc as bacc
import concourse.mybir as mybir
import concourse.tile as tile
import concourse.bass_utils as bass_utils
from concourse.masks import make_identity

B, S, M, D, NQ = 128, 100, 50, 128, 10000
NCORES = 8
BC = B // NCORES          # 16 batch rows per core
GB = 8                    # rows per group
GW = M * GB               # 400 group state width
NQTILES = (S * BC + 127) // 128   # 13 gather tiles
QCOLS = NQTILES * 128     # 1664
HQB = 8                   # steps per W1q@qe prefetch block

F32 = mybir.dt.float32
BF16 = mybir.dt.bfloat16
I32 = mybir.dt.int32
AF = mybir.ActivationFunctionType
OP = mybir.AluOpType
AX = mybir.AxisListType

_CACHE = {}


def _build_program():
    if "nc" in _CACHE:
        return _CACHE["nc"]

    nc = bacc.Bacc("TRN2", target_bir_lowering=False, debug=False,
                   enable_asserts=False, num_devices=NCORES)

    dram_in = {}
    for name, shape, dt in [
        ("qtb", [D, QCOLS], BF16),
        ("qlast", [D, BC], BF16),
        ("kTb", [D, M], BF16),
        ("w1r", [D, D], BF16), ("w1q", [D, D], BF16),
        ("w2er", [D, D], BF16), ("w2ad", [D, D], BF16),
        ("b1", [D, 1], F32), ("eb", [D, 1], F32), ("ab", [D, 1], F32),
        ("ow1r", [D, D], BF16), ("ow1q", [D, D], BF16),
        ("ob1", [D, 1], F32), ("ow2", [D, 1], BF16), ("ob2", [1, 1], F32),
    ]:
        dram_in[name] = nc.dram_tensor(name, shape, dt, kind="ExternalInput").ap()
    pred_out = nc.dram_tensor("pred", [1, BC], F32, kind="ExternalOutput").ap()

    with tile.TileContext(nc) as tc, ExitStack() as ctx:
        persist = ctx.enter_context(tc.tile_pool(name="persist", bufs=1))

        # ---- persistent SBUF tiles ----
        kTb = persist.tile([D, M], BF16, tag="kTb")
        w1r = persist.tile([D, D], BF16, tag="w1r")
        w1q = persist.tile([D, D], BF16, tag="w1q")
        w2er = persist.tile([D, D], BF16, tag="w2er")
        w2ad = persist.tile([D, D], BF16, tag="w2ad")
        b1 = persist.tile([D, 1], F32, tag="b1")
        eb = persist.tile([D, 1], F32, tag="eb")
        ab = persist.tile([D, 1], F32, tag="ab")
        ow1r = persist.tile([D, D], BF16, tag="ow1r")
        ow1q = persist.tile([D, D], BF16, tag="ow1q")
        ob1 = persist.tile([D, 1], F32, tag="ob1")
        ow2 = persist.tile([D, 1], BF16, tag="ow2")
        ob2 = persist.tile([1, 1], F32, tag="ob2")
        ident = persist.tile([128, 128], F32, tag="ident")
        identb = persist.tile([128, 128], BF16, tag="identb")
        qlast = persist.tile([D, BC], BF16, tag="qlast")
        qTb = persist.tile([D, QCOLS], BF16, tag="qTb")
        attn = persist.tile([S, 2 * GW], F32, tag="attn")
        attnb = persist.tile([S, 2 * GW], BF16, tag="attnb")
        vpp = [[persist.tile([D, GW], BF16, name=f"v{g}p{p}", tag=f"v{g}p{p}")
                for p in (0, 1)] for g in (0, 1)]

        # DMA priority: kTb first (phase 2 needs it with qtb), then qtb
        # split across the three DMA-capable queues, then the scan
        # weights, and the final-head weights (needed ~400us in) last.
        nc.sync.dma_start(kTb[:], dram_in["kTb"][:])
        TH = QCOLS // 3
        nc.sync.dma_start(qTb[:, 0:TH], dram_in["qtb"][:, 0:TH])
        nc.scalar.dma_start(qTb[:, TH:2 * TH], dram_in["qtb"][:, TH:2 * TH])
        nc.gpsimd.dma_start(qTb[:, 2 * TH:QCOLS],
                            dram_in["qtb"][:, 2 * TH:QCOLS])
        for i, (nm, t) in enumerate([
                ("w1r", w1r), ("w1q", w1q),
                ("w2er", w2er), ("w2ad", w2ad), ("b1", b1),
                ("eb", eb), ("ab", ab)]):
            eng = (nc.scalar, nc.gpsimd)[i % 2]
            eng.dma_start(t[:], dram_in[nm][:])
        for i, (nm, t) in enumerate([
                ("qlast", qlast), ("ow1r", ow1r),
                ("ow1q", ow1q), ("ob1", ob1), ("ow2", ow2),
                ("ob2", ob2)]):
            eng = (nc.scalar, nc.gpsimd)[i % 2]
            eng.dma_start(t[:], dram_in[nm][:])
        make_identity(nc, ident[:])
        nc.vector.tensor_copy(identb[:], ident[:])
        nc.vector.memset(vpp[0][0][:], 0.0)
        nc.vector.memset(vpp[1][0][:], 0.0)

        # ---- phase 2: scores + softmax -> attn[s, (b,m)] f32 ----
        with tc.tile_pool(name="spsum", bufs=4, space="PSUM") as spsum:
            for b in range(BC):
                sc = spsum.tile([S, M], F32, tag="sc")
                qTsl = qTb[:, b:S * BC:BC]        # [128, 100] strided (s,b)
                nc.tensor.matmul(sc[:], qTsl, kTb[:], start=True, stop=True)
                if b % 2 == 0:
                    nc.vector.tensor_copy(attn[:, b * M:(b + 1) * M], sc[:])
                else:
                    nc.scalar.copy(attn[:, b * M:(b + 1) * M], sc[:])

        with tc.tile_pool(name="smx", bufs=1) as smx:
            a3 = attn[:].rearrange("p (b m) -> p b m", b=BC)
            # No max-subtraction: scores are O(1) here (xavier embeddings,
            # D=128), so raw exp is safe in f32.
            nc.scalar.activation(attn[:], attn[:], AF.Exp)
            sm = smx.tile([S, BC], F32, tag="sm")
            nc.vector.tensor_reduce(sm[:], a3, axis=AX.X, op=OP.add)
            rec = smx.tile([S, BC], F32, tag="rec")
            nc.vector.reciprocal(rec[:], sm[:])
            # normalize fused with the (b,m)->(m,b) bf16 reshuffle
            src = attn[:].rearrange("p (g b m) -> p g b m", g=2, b=GB)
            dst = attnb[:].rearrange("p (g m b) -> p g b m", g=2, m=M)
            recb = rec[:].rearrange("p (g b) -> p g b", g=2)[:, :, :, None] \
                .broadcast_to([S, 2, GB, M])
            nc.vector.tensor_tensor(dst, src, recb, op=OP.mult)
            # Preload the Tanh/Sigmoid activation table now (dead time)
            # instead of stalling the scan's first gate for ~1.3us.
            dmy = smx.tile([1, 1], F32, tag="dmy")
            nc.scalar.activation(dmy[:], rec[0:1, 0:1], AF.Tanh)

        # ---- phase 3: the scan ----
        # PSUM banks: wps 4 bufs x 1 bank + hqp 2 x 1 + mlpp 2 x 1 = 8.
        with tc.tile_pool(name="wps", bufs=4, space="PSUM") as wps, \
             tc.tile_pool(name="hqp", bufs=2, space="PSUM") as hqp, \
             tc.tile_pool(name="mlp", bufs=2, space="PSUM") as mlpp, \
             tc.tile_pool(name="w8p", bufs=8) as w8p, \
             tc.tile_pool(name="zp", bufs=4) as zp, \
             tc.tile_pool(name="wide", bufs=9) as wide, \
             tc.tile_pool(name="small", bufs=16) as small:

            NS = 2 * S  # slots
            wtile = [None] * NS     # per-slot w_sb bf16 slice APs
            hqt = {}                # step-block k0 -> PSUM tile
            state = [None] * NS     # (z, w, e, a) per slot
            wcopy_q = []            # slots whose w copy is pending

            wpsum = [None] * NS     # per-slot PSUM broadcast tiles

            def emit_w_mm(s):
                # One slot's w broadcast: one-hot row select via PE into a
                # 1-bank PSUM tile. The bf16 ACT copy (emit_w_copy) is
                # emitted a couple of iterations later so the ~585ns copy
                # lump never stalls a just-ready gate for long.
                t, g = s // 2, s % 2
                sel = identb[0:S, t:t + 1].broadcast_to([S, D])
                wp = wps.tile([D, GW], F32, tag="wp")
                nc.tensor.matmul(wp[:], sel, attnb[:, g * GW:(g + 1) * GW],
                                 start=True, stop=True)
                wpsum[s] = wp

            def emit_w_copy(s):
                w8 = w8p.tile([D, GW], BF16, tag="w8")
                nc.scalar.copy(w8[:], wpsum[s][:])
                wpsum[s] = None
                wtile[s] = w8[:]

            def emit_hq_block(k0):
                # W1q @ qe for steps [k0, k0+HQB): one 128-col matmul.
                # Per-slot read matmuls later accumulate into 8-col
                # slices (start=False), then ACT tanh reads the slice.
                ke = min(k0 + HQB, S)
                hq = hqp.tile([D, HQB * BC], F32, tag="hq")
                nc.tensor.matmul(hq[:, 0:(ke - k0) * BC], w1q[:],
                                 qTb[:, k0 * BC:ke * BC],
                                 start=True, stop=False,
                                 skip_group_check=True)
                hqt[k0] = hq

            def emit_read_gates(s):
                t, g = s // 2, s % 2
                w = wtile[s]
                vcur = vpp[g][t % 2]
                k0 = (t // HQB) * HQB
                hq = hqt[k0]
                hsl = hq[:, (t - k0) * BC + g * GB:(t - k0) * BC + (g + 1) * GB]
                # z full-width, then one fused read matmul:
                # W1r @ read = sum_m (W1r @ z[:,m]) via stride-0 PSUM output
                # AP (start=False accumulates by address) on top of the
                # prefetched W1q@qe slice.
                z = zp.tile([D, GW], BF16, tag="z")
                nc.vector.tensor_tensor(z[:], vcur[:], w, op=OP.mult)
                hbc = hsl[:, None, :].broadcast_to([D, M, GB])
                nc.tensor.matmul(
                    hbc, w1r[:],
                    z[:].rearrange("p (m b) -> p m b", m=M),
                    start=False, stop=True, skip_group_check=True)
                th = small.tile([D, GB], BF16, tag="th")
                nc.scalar.activation(th[:], hsl, AF.Tanh, bias=b1[:])
                # The prefetched w copy goes right here in the Scalar
                # queue: while e-mm does its PE round trip (~430ns), ACT
                # would idle anyway, so the copy lump can't delay e-act by
                # much.
                if wcopy_q:
                    emit_w_copy(wcopy_q.pop(0))
                # e and a get separate PSUM tiles (same tag ring): a shared
                # tile makes the second matmul wait on the first gate's ACT
                # read (tile-granular WAR), adding ~500ns to the chain.
                eps = mlpp.tile([D, GB], F32, tag="ea")
                nc.tensor.matmul(eps[:], w2er[:], th[:], start=True,
                                 stop=True)
                e = small.tile([D, GB], BF16, tag="e")
                nc.scalar.activation(e[:], eps[:], AF.Sigmoid, bias=eb[:])
                aps = mlpp.tile([D, GB], F32, tag="ea")
                nc.tensor.matmul(aps[:], w2ad[:], th[:], start=True,
                                 stop=True)
                a = small.tile([D, GB], BF16, tag="a")
                nc.scalar.activation(a[:], aps[:], AF.Tanh, bias=ab[:])
                state[s] = (z, w, e, a)

            def emit_update(s):
                t, g = s // 2, s % 2
                z, w, e, a = state[s]
                vcur, vnext = vpp[g][t % 2], vpp[g][(t + 1) % 2]
                # All DVE, ordered t1, p, t2, v': t1/p depend only on e
                # (which lands ~270ns before a), and by the time p is done
                # a has long arrived, so the queue never stalls mid-tail.
                # (Putting any of these on Pool was tried three ways and
                # always regressed: Pool's launch+sem latency is poison
                # anywhere near the chain.)
                ebc = e[:, None, :].broadcast_to([D, M, GB])
                t1 = wide.tile([D, GW], BF16, tag="t1")
                nc.vector.tensor_tensor(
                    t1[:].rearrange("p (m b) -> p m b", m=M),
                    z[:].rearrange("p (m b) -> p m b", m=M),
                    ebc, op=OP.mult)
                p = wide.tile([D, GW], BF16, tag="p")
                nc.vector.tensor_tensor(p[:], vcur[:], t1[:], op=OP.subtract)
                abc = a[:, None, :].broadcast_to([D, M, GB])
                t2 = wide.tile([D, GW], BF16, tag="t2")
                nc.vector.tensor_tensor(
                    t2[:].rearrange("p (m b) -> p m b", m=M),
                    w.rearrange("p (m b) -> p m b", m=M),
                    abc, op=OP.mult)
                nc.vector.tensor_tensor(vnext[:], p[:], t2[:], op=OP.add)
                state[s] = None

            # Anti-phase slot order (0, 2, 1, 4, 3, ...): group 0 runs one
            # step ahead of group 1 so each group's gate latency is hidden
            # under the other group's DVE stretch instead of bunching.
            order = [0] + [x for k in range(1, S) for x in (2 * k, 2 * k - 1)] \
                + [NS - 1]
            next_hq = [0]

            def ensure_hq(s):
                while next_hq[0] <= s // 2 and next_hq[0] < S:
                    emit_hq_block(next_hq[0])
                    next_hq[0] += HQB

            ensure_hq(order[0])
            for j in range(3):
                emit_w_mm(order[j])
            for j in range(2):
                emit_w_copy(order[j])
            for i, s in enumerate(order):
                if s >= 2:
                    emit_update(s - 2)
                if i + 2 < NS:
                    ensure_hq(order[i + 2])
                emit_read_gates(s)
                if i + 3 < NS:
                    emit_w_mm(order[i + 3])
                if i + 2 < NS:
                    wcopy_q.append(order[i + 2])
            while wcopy_q:
                emit_w_copy(wcopy_q.pop(0))
            emit_update(NS - 2)
            emit_update(NS - 1)

            # ---- final prediction (uses w from t=S-1, v after last update) ----
            # Final read via the same PE identity-accumulate trick as the
            # per-step read (stride-0 PSUM out AP sums the 50 m-slices);
            # beats two serial 840ns DVE tensor_reduces.
            rfps = mlpp.tile([D, BC], F32, tag="ea")
            nc.vector.memset(rfps[:], 0.0)
            for g in (0, 1):
                wf = wtile[2 * (S - 1) + g]
                zfin = zp.tile([D, GW], BF16, tag="z")
                nc.vector.tensor_tensor(zfin[:], vpp[g][S % 2][:], wf,
                                        op=OP.mult)
                rbc = rfps[:, None, g * GB:(g + 1) * GB] \
                    .broadcast_to([D, M, GB])
                nc.tensor.matmul(rbc, identb[:],
                                 zfin[:].rearrange("p (m b) -> p m b", m=M),
                                 start=False, stop=(g == 1),
                                 skip_group_check=True)
            readF = small.tile([D, BC], BF16, tag="readF")
            nc.scalar.copy(readF[:], rfps[:])
            h2ps = mlpp.tile([D, BC], F32, tag="ea")
            nc.tensor.matmul(h2ps[:], ow1r[:], readF[:], start=True, stop=False)
            nc.tensor.matmul(h2ps[:], ow1q[:], qlast[:], start=False, stop=True)
            h2 = small.tile([D, BC], BF16, tag="h2")
            nc.scalar.activation(h2[:], h2ps[:], AF.Relu, bias=ob1[:])
            pps = mlpp.tile([D, BC], F32, tag="ea")
            nc.tensor.matmul(pps[0:1, 0:BC], ow2[:], h2[:], start=True,
                             stop=True)
            ps = small.tile([1, BC], F32, tag="pred")
            nc.scalar.activation(ps[:], pps[0:1, 0:BC], AF.Sigmoid, bias=ob2[:])
            nc.sync.dma_start(pred_out[:], ps[:])

    nc.compile()
    _CACHE["nc"] = nc
    return nc


def _host_inputs(inputs):
    """Per-core input maps from the full problem inputs."""
    q = np.asarray(inputs["question_seq"]).astype(np.int64)
    emb = np.ascontiguousarray(np.asarray(inputs["emb"], dtype=np.float32))
    key_matrix = np.asarray(inputs["key_matrix"], dtype=np.float32)
    vu_w1 = np.asarray(inputs["vu_w1"], dtype=np.float32)
    vu_b1 = np.asarray(inputs["vu_b1"], dtype=np.float32)
    vu_w2 = np.asarray(inputs["vu_w2"], dtype=np.float32)
    vu_b2 = np.asarray(inputs["vu_b2"], dtype=np.float32)
    er_w = np.asarray(inputs["er_w"], dtype=np.float32)
    er_b = np.asarray(inputs["er_b"], dtype=np.float32)
    ad_w = np.asarray(inputs["ad_w"], dtype=np.float32)
    ad_b = np.asarray(inputs["ad_b"], dtype=np.float32)
    out_w1 = np.asarray(inputs["out_w1"], dtype=np.float32)
    out_b1 = np.asarray(inputs["out_b1"], dtype=np.float32)
    out_w2 = np.asarray(inputs["out_w2"], dtype=np.float32)
    out_b2 = np.asarray(inputs["out_b2"], dtype=np.float32)

    w2er = (vu_w2.astype(np.float64) @ er_w.astype(np.float64)).astype(np.float32)
    w2ad = (vu_w2.astype(np.float64) @ ad_w.astype(np.float64)).astype(np.float32)
    ebf = (vu_b2.astype(np.float64) @ er_w.astype(np.float64) + er_b).astype(np.float32)
    abf = (vu_b2.astype(np.float64) @ ad_w.astype(np.float64) + ad_b).astype(np.float32)

    bf = ml_dtypes.bfloat16
    shared = {
        "kTb": np.ascontiguousarray(key_matrix.T).astype(bf),
        "w1r": np.ascontiguousarray(vu_w1[:D]).astype(bf),
        "w1q": np.ascontiguousarray(vu_w1[D:]).astype(bf),
        "w2er": w2er.astype(bf), "w2ad": w2ad.astype(bf),
        "b1": vu_b1.reshape(D, 1), "eb": ebf.reshape(D, 1), "ab": abf.reshape(D, 1),
        "ow1r": np.ascontiguousarray(out_w1[:D]).astype(bf),
        "ow1q": np.ascontiguousarray(out_w1[D:]).astype(bf),
        "ob1": out_b1.reshape(D, 1),
        "ow2": np.ascontiguousarray(out_w2.reshape(D, 1)).astype(bf),
        "ob2": out_b2.reshape(1, 1),
    }
    in_maps = []
    for c in range(NCORES):
        qc = q[c * BC:(c + 1) * BC, :]          # [BC, S]
        idxs = qc.T.reshape(-1)                  # n = s*BC + b order
        qg = emb[idxs]                           # [S*BC, D]
        qtb = np.zeros((D, QCOLS), np.float32)
        qtb[:, :S * BC] = qg.T
        m = dict(shared)
        m["qtb"] = qtb.astype(bf)
        m["qlast"] = np.ascontiguousarray(qg[(S - 1) * BC:, :].T).astype(bf)
        in_maps.append(m)
    return in_maps


def _install_ntff_shim():
    # Optional: enables NTFF hardware profiling under axon when tracing is
    # requested. Harmless no-op if the pieces are missing.
    import types, sys
    if "antenv.axon_hooks" in sys.modules:
        return
    try:
        import antenv
        from trn_agent_boot.trn_boot import _ntff_profile_via_ctypes
        hook = _ntff_profile_via_ctypes("/opt/axon/libaxon_pjrt.so")
        mod = types.ModuleType("antenv.axon_hooks")
        state = {"hook": hook}
        mod.get_axon_ntff_profile_hook = lambda: state["hook"]
        mod.set_axon_ntff_profile_hook = lambda h: state.update(hook=h)
        sys.modules["antenv.axon_hooks"] = mod
        antenv.axon_hooks = mod
    except Exception:
        pass


def kernel(**inputs) -> np.ndarray:
    if bool(int(os.environ.get("DKVMN_TRACE", "0"))):
        _install_ntff_shim()
    nc = _build_program()
    in_maps = _host_inputs(inputs)
    res = bass_utils.run_bass_kernel_spmd(
        nc, in_maps, core_ids=list(range(NCORES)),
        trace=bool(int(os.environ.get("DKVMN_TRACE", "0"))),
    )
    _CACHE["last_results"] = res
    pred = np.concatenate([res.results[c]["pred"].reshape(BC) for c in range(NCORES)])
    return pred.astype(np.float32)
